# revision 11
# baseline (speedup 1.0000x reference)
"""DGCNN forward kernel for Trainium2, data-parallel over 8 NeuronCores.

Strategy (per core, 2 point clouds):
  - EdgeConv layer l: h[n,k,:] = [x_n, x_j - x_n] @ W + b decomposes as
        A[n] = x_n @ (W_top - W_bot) + b          (per-node, PE matmul)
        Bm[j] = x_j @ W_bot                       (per-node, PE matmul)
        out[n] = A[n] + max_{j in knn(n)} Bm[j]
  - kNN ranking metric s[n,j] = 2*x_n.x_j - |x_j|^2 (row-monotone with -dist),
    computed on PE; top-20 per row via DVE max/max_index/match_replace.
  - Neighbor max via 20 chained indirect DMA gathers (DRAM Bm rows) with
    CCE accumulate-max into SBUF.
  - cat -> Wl matmul per 128-channel tile (feature-major); per-channel
    sum / sumsq (ScalarE accum) and per-graph max over nodes (DVE reduce)
    are the only outputs: batch-norm over all B*N rows + leaky relu commute
    with the max-pool (positive scale), so the tiny tail (BN affine on the
    16x1024 pooled matrix + 2-layer classifier) runs on host.
"""

import os
import sys
from contextlib import ExitStack

for _p in ("/opt/trn_rl_repo", "/root/.axon_site/_ro/trn_rl_repo"):
    if os.path.isdir(_p) and _p not in sys.path:
        sys.path.insert(0, _p)

import numpy as np

import concourse.bacc as bacc
import concourse.bass as bass
import concourse.tile as tile
from concourse import mybir
from concourse.bass import IndirectOffsetOnAxis
from concourse.bass_utils import run_bass_kernel_spmd
from concourse.masks import make_identity

F32 = mybir.dt.float32
U32 = mybir.dt.uint32
U16 = mybir.dt.uint16
I16 = mybir.dt.int16

B, N = 16, 1024
NCORES = 8
GPC = B // NCORES  # graphs per core
# (d_in, c_out) for the 4 EdgeConv layers
LAYERS = [(3, 64), (64, 64), (64, 128), (128, 256)]
LIN_DIM, HID = 1024, 512
EPS = 1e-5
NEG = 0.2
NEG_INF = -1.0e30


def _build(nc, k):
    rounds = (k + 7) // 8
    kslots = rounds * 8

    posT = nc.dram_tensor("posT", [GPC, 3, N], F32, kind="ExternalInput").ap()
    wbot, wd, bias = [], [], []
    for li, (d, c) in enumerate(LAYERS):
        wbot.append(nc.dram_tensor(f"wbot{li}", [d, c], F32, kind="ExternalInput").ap())
        wd.append(nc.dram_tensor(f"wd{li}", [d, c], F32, kind="ExternalInput").ap())
        bias.append(nc.dram_tensor(f"b{li}", [1, c], F32, kind="ExternalInput").ap())
    wl = nc.dram_tensor("wl", [128, 4, LIN_DIM], F32, kind="ExternalInput").ap()

    s1o = nc.dram_tensor("s1", [128, 32], F32, kind="ExternalOutput").ap()
    s2o = nc.dram_tensor("s2", [128, 32], F32, kind="ExternalOutput").ap()
    mxo = nc.dram_tensor("mx", [128, 32], F32, kind="ExternalOutput").ap()

    with tile.TileContext(nc) as tc, ExitStack() as ctx:
        consts = ctx.enter_context(tc.tile_pool(name="consts", bufs=1))
        sb = ctx.enter_context(tc.tile_pool(name="sb", bufs=2))
        xtp = ctx.enter_context(tc.tile_pool(name="xt", bufs=2))
        spool = ctx.enter_context(tc.tile_pool(name="sp", bufs=2))
        gpool = ctx.enter_context(tc.tile_pool(name="gp", bufs=2))
        opool = ctx.enter_context(tc.tile_pool(name="op", bufs=2))
        statp = ctx.enter_context(tc.tile_pool(name="stat", bufs=1))
        psum_s = ctx.enter_context(tc.tile_pool(name="psum_s", bufs=2, space="PSUM"))
        psum_m = ctx.enter_context(tc.tile_pool(name="psum_m", bufs=2, space="PSUM"))
        dram = ctx.enter_context(tc.tile_pool(name="dram", bufs=2, space="DRAM"))

        ones = consts.tile([128, 512], F32)
        nc.gpsimd.memset(ones[:], 1.0)

        wbotS, wdS, bS = [], [], []
        for li, (d, c) in enumerate(LAYERS):
            wb = consts.tile([d, c], F32, tag=f"wbot{li}")
            nc.sync.dma_start(out=wb[:], in_=wbot[li])
            wbotS.append(wb)
            wdt = consts.tile([d, c], F32, tag=f"wd{li}")
            nc.sync.dma_start(out=wdt[:], in_=wd[li])
            wdS.append(wdt)
            bt = consts.tile([1, c], F32, tag=f"b{li}")
            nc.sync.dma_start(out=bt[:], in_=bias[li])
            bS.append(bt)
        wlS = consts.tile([128, 4, LIN_DIM], F32)
        nc.sync.dma_start(out=wlS[:], in_=wl)

        s1p = statp.tile([128, 32], F32)
        s2p = statp.tile([128, 32], F32)
        mxp = statp.tile([128, 32], F32)

        # k-slot groups for the gather (groups of up to 5 neighbor slots)
        kgroups = []
        k0 = 0
        while k0 < k:
            m = min(5, k - k0)
            kgroups.append((k0, m))
            k0 += m

        for g in range(GPC):
            x0 = sb.tile([3, N], F32, tag="rowbuf")
            nc.sync.dma_start(out=x0[:], in_=posT[g])
            catT = xtp.tile([128, 4, N], F32, tag="catT")
            prev_edge = None

            for li, (d, c) in enumerate(LAYERS):
                ncb = (c + 127) // 128
                if li == 0:
                    xT = x0[:, :]
                else:
                    xT = prev_edge[0:d, 0, :]

                # x2T = 2*x.T ; xsq = x.T^2 ; negsq[j] = -sum_d x[j,d]^2
                x2T = sb.tile([d, N], F32, tag="x2T")
                nc.scalar.mul(out=x2T[:], in_=xT, mul=2.0)
                xsq = sb.tile([d, N], F32, tag="scr")
                nc.scalar.activation(
                    out=xsq[:], in_=xT, func=mybir.ActivationFunctionType.Square
                )
                negsq = sb.tile([1, N], F32, tag="rowbuf")
                for h in range(2):
                    qp = psum_m.tile([128, 512], F32, tag="pm")
                    nc.tensor.matmul(
                        qp[0:1, :],
                        lhsT=ones[0:d, 0:1],
                        rhs=xsq[:, h * 512 : (h + 1) * 512],
                        start=True,
                        stop=True,
                    )
                    nc.scalar.mul(
                        out=negsq[:, h * 512 : (h + 1) * 512], in_=qp[0:1, :], mul=-1.0
                    )

                # BmT = (x @ W_bot).T and AT = (x @ Wd + b).T, feature-major
                bmT = sb.tile([128, 2, N], F32, tag="bmT")
                aT = sb.tile([128, 2, N], F32, tag="aT")
                for cb in range(ncb):
                    cw = min(128, c - cb * 128)
                    cs = slice(cb * 128, cb * 128 + cw)
                    for h in range(2):
                        hs = slice(h * 512, (h + 1) * 512)
                        bp = psum_m.tile([128, 512], F32, tag="pm")
                        nc.tensor.matmul(
                            bp[0:cw, :], lhsT=wbotS[li][:, cs], rhs=xT[:, hs],
                            start=True, stop=True,
                        )
                        nc.scalar.copy(out=bmT[0:cw, cb, hs], in_=bp[0:cw, :])
                        ap_ = psum_m.tile([128, 512], F32, tag="pm")
                        nc.tensor.matmul(
                            ap_[0:cw, :], lhsT=wdS[li][:, cs], rhs=xT[:, hs],
                            start=True, stop=False,
                        )
                        nc.tensor.matmul(
                            ap_[0:cw, :], lhsT=bS[li][:, cs], rhs=ones[0:1, 0:512],
                            start=False, stop=True,
                        )
                        nc.scalar.copy(out=aT[0:cw, cb, hs], in_=ap_[0:cw, :])

                # distance metric + top-k indices
                O = opool.tile([128, kslots, 8], U16, tag="O")
                for q in range(8):
                    xs = xT[:, q * 128 : (q + 1) * 128]
                    sp = psum_s.tile([128, N], F32, tag="s")
                    for h in range(2):
                        sl = slice(h * 512, (h + 1) * 512)
                        nc.tensor.matmul(
                            sp[:, sl], lhsT=xs, rhs=x2T[:, sl], start=True, stop=False
                        )
                        nc.tensor.matmul(
                            sp[:, sl],
                            lhsT=ones[0:1, 0:128],
                            rhs=negsq[:, sl],
                            start=False,
                            stop=True,
                        )
                    S = spool.tile([128, N], F32, tag="S")
                    nc.scalar.copy(out=S[:], in_=sp[:])
                    mx8 = sb.tile([128, 8], F32, tag="mx8")
                    for r in range(rounds):
                        nc.vector.max(out=mx8[:], in_=S[:])
                        nc.vector.max_index(
                            out=O[:, r * 8 : (r + 1) * 8, q], in_max=mx8[:], in_values=S[:]
                        )
                        if r < rounds - 1:
                            nc.vector.match_replace(
                                out=S[:], in_to_replace=mx8[:], in_values=S[:],
                                imm_value=NEG_INF,
                            )

                # index wrap: O[p, k, q] -> k-major list in 16-partition wrap
                idxD = dram.tile([kslots, N], U16, tag="idxD")
                nc.sync.dma_start(
                    out=idxD[:].rearrange("kk (q p) -> p kk q", p=128), in_=O[:]
                )
                idx16 = sb.tile([128, kslots * 64], I16, tag="idx16")
                idxDv = idxD[:].rearrange("kk (a r) -> r (kk a)", r=16).bitcast(I16)
                for gr in range(8):
                    nc.sync.dma_start(
                        out=idx16[16 * gr : 16 * (gr + 1), :], in_=idxDv
                    )

                # gather neighbor BmT columns per k-group, max-reduce on DVE
                edge = gpool.tile([128, 2, N], F32, tag="edge")
                for cb in range(ncb):
                    cw = min(128, c - cb * 128)
                    e = edge[0:cw, cb, :]
                    first = True
                    for (kk0, m) in kgroups:
                        ping = gpool.tile([128, 5 * N], F32, tag="ping")
                        nc.gpsimd.ap_gather(
                            out_ap=ping[0:cw, 0 : m * N],
                            in_ap=bmT[0:cw, cb, :],
                            idxs_ap=idx16[0:cw, kk0 * 64 : (kk0 + m) * 64],
                            channels=cw,
                            num_elems=N,
                            d=1,
                            num_idxs=m * N,
                        )
                        j0 = 0
                        if first:
                            nc.vector.tensor_max(
                                out=e, in0=ping[0:cw, 0:N], in1=ping[0:cw, N : 2 * N]
                            )
                            j0 = 2
                            first = False
                        for j in range(j0, m):
                            nc.vector.tensor_max(
                                out=e, in0=e, in1=ping[0:cw, j * N : (j + 1) * N]
                            )
                    # edge = A.T + max
                    nc.vector.tensor_add(out=e, in0=e, in1=aT[0:cw, cb, :])

                # copy into catT rows (DMA handles partition shifts)
                if li == 0:
                    nc.sync.dma_start(out=catT[0:64, 0, :], in_=edge[0:64, 0, :])
                elif li == 1:
                    nc.sync.dma_start(out=catT[64:128, 0, :], in_=edge[0:64, 0, :])
                elif li == 2:
                    nc.sync.dma_start(out=catT[:, 1, :], in_=edge[:, 0, :])
                else:
                    nc.sync.dma_start(out=catT[:, 2, :], in_=edge[:, 0, :])
                    nc.sync.dma_start(out=catT[:, 3, :], in_=edge[:, 1, :])
                prev_edge = edge

            # linear layer, stats only
            for ct in range(8):
                for nh in range(2):
                    lp = psum_m.tile([128, 512], F32, tag="pm")
                    for kc in range(4):
                        nc.tensor.matmul(
                            lp[:],
                            lhsT=wlS[:, kc, ct * 128 : (ct + 1) * 128],
                            rhs=catT[:, kc, nh * 512 : (nh + 1) * 512],
                            start=(kc == 0),
                            stop=(kc == 3),
                        )
                    col = ct * 4 + nh * 2 + g
                    scr = sb.tile([128, 512], F32, tag="scr")
                    nc.scalar.activation(
                        out=scr[:], in_=lp[:],
                        func=mybir.ActivationFunctionType.Square,
                        accum_out=s2p[:, col : col + 1],
                    )
                    scr2 = sb.tile([128, 512], F32, tag="scr")
                    nc.scalar.activation(
                        out=scr2[:], in_=lp[:],
                        func=mybir.ActivationFunctionType.Copy,
                        accum_out=s1p[:, col : col + 1],
                    )
                    nc.vector.tensor_reduce(
                        out=mxp[:, col : col + 1], in_=lp[:],
                        axis=mybir.AxisListType.X, op=mybir.AluOpType.max,
                    )

        nc.sync.dma_start(out=s1o, in_=s1p[:])
        nc.sync.dma_start(out=s2o, in_=s2p[:])
        nc.sync.dma_start(out=mxo, in_=mxp[:])
    return nc


def _prep_inputs(pos, k, weights):
    """Build the per-core in_maps."""
    pos = np.asarray(pos, dtype=np.float32)
    in_maps = []
    shared = {}
    for li, (d, c) in enumerate(LAYERS):
        W = np.asarray(weights[f"W{li + 1}"], dtype=np.float32)
        b = np.asarray(weights[f"b{li + 1}"], dtype=np.float32)
        wtop, wbot = W[:d], W[d:]
        shared[f"wbot{li}"] = np.ascontiguousarray(wbot)
        shared[f"wd{li}"] = np.ascontiguousarray(wtop - wbot)
        shared[f"b{li}"] = np.ascontiguousarray(b[None, :])
    Wl = np.asarray(weights["Wl"], dtype=np.float32)
    shared["wl"] = np.ascontiguousarray(
        Wl.reshape(4, 128, LIN_DIM).transpose(1, 0, 2)
    )
    for core in range(NCORES):
        m = dict(shared)
        sl = pos[core * GPC : (core + 1) * GPC]  # [GPC, N, 3]
        m["posT"] = np.ascontiguousarray(sl.transpose(0, 2, 1))
        in_maps.append(m)
    return in_maps


def _leaky(x):
    return np.where(x > 0, x, np.float32(NEG) * x).astype(np.float32)


def _host_tail(results, inputs):
    """Combine per-core stats and run the tiny network tail on host."""
    bl = np.asarray(inputs["bl"], np.float32)
    gl = np.asarray(inputs["gl"], np.float32)
    betal = np.asarray(inputs["betal"], np.float32)
    Wm1 = np.asarray(inputs["Wm1"], np.float32)
    bm1 = np.asarray(inputs["bm1"], np.float32)
    gm1 = np.asarray(inputs["gm1"], np.float32)
    betam1 = np.asarray(inputs["betam1"], np.float32)
    Wm2 = np.asarray(inputs["Wm2"], np.float32)
    bm2 = np.asarray(inputs["bm2"], np.float32)

    S1 = np.zeros(LIN_DIM, np.float64)
    S2 = np.zeros(LIN_DIM, np.float64)
    M = np.full((B, LIN_DIM), -np.inf, np.float32)
    for core in range(NCORES):
        r = results[core]
        s1, s2, mx = r["s1"], r["s2"], r["mx"]  # [128, 32]
        for ct in range(8):
            cs = slice(ct * 128, (ct + 1) * 128)
            for nh in range(2):
                for g in range(GPC):
                    col = ct * 4 + nh * 2 + g
                    S1[cs] += s1[:, col].astype(np.float64)
                    S2[cs] += s2[:, col].astype(np.float64)
                    gi = core * GPC + g
                    M[gi, cs] = np.maximum(M[gi, cs], mx[:, col])

    # rows of lin_pre are x + bl; BN over axis 0 is shift invariant in bl only
    # through (x - m); the max-pool then needs the bl-shifted values:
    n_rows = B * N
    mean = (S1 / n_rows).astype(np.float32) + bl
    var = (S2 / n_rows - (S1 / n_rows) ** 2).astype(np.float32)
    Mb = M + bl[None, :]
    rstd = (1.0 / np.sqrt(var + np.float32(EPS))).astype(np.float32)
    pooled = _leaky(gl * (Mb - mean[None, :]) * rstd + betal)

    h = pooled @ Wm1 + bm1
    m1 = h.mean(axis=0, dtype=np.float32)
    v1 = h.var(axis=0, dtype=np.float32)
    h = _leaky(gm1 * (h - m1) * (1.0 / np.sqrt(v1 + np.float32(EPS))) + betam1)
    return (h @ Wm2 + bm2).astype(np.float32)


def run_device(inputs, trace=False):
    k = int(inputs["k"])
    nc = bacc.Bacc(
        "TRN2",
        target_bir_lowering=False,
        debug=False,
        enable_asserts=False,
        num_devices=NCORES,
    )
    _build(nc, k)
    nc.compile()
    in_maps = _prep_inputs(inputs["pos"], k, inputs)
    kw = {}
    if trace:
        _register_ntff_hook()
        kw = dict(trace=True)
    res = run_bass_kernel_spmd(nc, in_maps, list(range(NCORES)), **kw)
    return res


def kernel(**inputs):
    res = run_device(inputs, trace=False)
    return _host_tail(res.results, inputs)


def _register_ntff_hook():
    """Register the axon NTFF profiling hook (antenv.axon_hooks is absent in
    this image; recreate the minimal module so trace=True can capture)."""
    import contextlib
    import ctypes
    import types

    name = "antenv.axon_hooks"
    if name in sys.modules:
        return
    so_path = "/opt/axon/libaxon_pjrt.so"
    mod = types.ModuleType(name)
    _hook_holder = [None]

    try:
        lib = ctypes.CDLL(so_path)
        lib.axon_start_nrt_profile.argtypes = [
            ctypes.POINTER(ctypes.c_int64),
            ctypes.c_size_t,
        ]
        lib.axon_start_nrt_profile.restype = ctypes.c_int64
        lib.axon_stop_nrt_profile.argtypes = [ctypes.c_char_p]
        lib.axon_stop_nrt_profile.restype = ctypes.c_int64

        @contextlib.contextmanager
        def _hook(output_dir, device_ids):
            import jax

            jax.devices()
            if device_ids:
                ids = (ctypes.c_int64 * len(device_ids))(*device_ids)
                rc = lib.axon_start_nrt_profile(ids, len(device_ids))
            else:
                rc = lib.axon_start_nrt_profile(None, 0)
            if rc != 0:
                raise RuntimeError(f"axon_start_nrt_profile rc={rc}")
            try:
                yield
            finally:
                n = lib.axon_stop_nrt_profile(str(output_dir).encode())
                print(f"ntff profile: {n} file(s) -> {output_dir}", file=sys.stderr)

        _hook_holder[0] = _hook
    except Exception as e:  # noqa: BLE001
        print(f"ntff hook unavailable: {e}", file=sys.stderr)

    mod.get_axon_ntff_profile_hook = lambda: _hook_holder[0]
    mod.set_axon_ntff_profile_hook = lambda h: _hook_holder.__setitem__(0, h)
    sys.modules[name] = mod


# revision 12
# speedup vs baseline: 1.0310x; 1.0310x over previous
"""DGCNN forward kernel for Trainium2, data-parallel over 8 NeuronCores.

Strategy (per core, 2 point clouds):
  - EdgeConv layer l: h[n,k,:] = [x_n, x_j - x_n] @ W + b decomposes as
        A[n] = x_n @ (W_top - W_bot) + b          (per-node, PE matmul)
        Bm[j] = x_j @ W_bot                       (per-node, PE matmul)
        out[n] = A[n] + max_{j in knn(n)} Bm[j]
  - kNN ranking metric s[n,j] = 2*x_n.x_j - |x_j|^2 (row-monotone with -dist),
    computed on PE; top-20 per row via DVE max/max_index/match_replace.
  - Neighbor max via 20 chained indirect DMA gathers (DRAM Bm rows) with
    CCE accumulate-max into SBUF.
  - cat -> Wl matmul per 128-channel tile (feature-major); per-channel
    sum / sumsq (ScalarE accum) and per-graph max over nodes (DVE reduce)
    are the only outputs: batch-norm over all B*N rows + leaky relu commute
    with the max-pool (positive scale), so the tiny tail (BN affine on the
    16x1024 pooled matrix + 2-layer classifier) runs on host.
"""

import os
import sys
from contextlib import ExitStack

for _p in ("/opt/trn_rl_repo", "/root/.axon_site/_ro/trn_rl_repo"):
    if os.path.isdir(_p) and _p not in sys.path:
        sys.path.insert(0, _p)

import numpy as np

import concourse.bacc as bacc
import concourse.bass as bass
import concourse.tile as tile
from concourse import mybir
from concourse.bass import IndirectOffsetOnAxis
from concourse.bass_utils import run_bass_kernel_spmd
from concourse.masks import make_identity

F32 = mybir.dt.float32
U32 = mybir.dt.uint32
U16 = mybir.dt.uint16
I16 = mybir.dt.int16

B, N = 16, 1024
NCORES = 8
GPC = B // NCORES  # graphs per core
# (d_in, c_out) for the 4 EdgeConv layers
LAYERS = [(3, 64), (64, 64), (64, 128), (128, 256)]
LIN_DIM, HID = 1024, 512
EPS = 1e-5
NEG = 0.2
NEG_INF = -1.0e30


def _build(nc, k):
    rounds = (k + 7) // 8
    kslots = rounds * 8

    posT = nc.dram_tensor("posT", [GPC, 3, N], F32, kind="ExternalInput").ap()
    wbot, wd, bias = [], [], []
    for li, (d, c) in enumerate(LAYERS):
        wbot.append(nc.dram_tensor(f"wbot{li}", [d, c], F32, kind="ExternalInput").ap())
        wd.append(nc.dram_tensor(f"wd{li}", [d, c], F32, kind="ExternalInput").ap())
        bias.append(nc.dram_tensor(f"b{li}", [1, c], F32, kind="ExternalInput").ap())
    wl = nc.dram_tensor("wl", [128, 4, LIN_DIM], F32, kind="ExternalInput").ap()

    s1o = nc.dram_tensor("s1", [128, 32], F32, kind="ExternalOutput").ap()
    s2o = nc.dram_tensor("s2", [128, 32], F32, kind="ExternalOutput").ap()
    mxo = nc.dram_tensor("mx", [128, 32], F32, kind="ExternalOutput").ap()

    with tile.TileContext(nc) as tc, ExitStack() as ctx:
        consts = ctx.enter_context(tc.tile_pool(name="consts", bufs=1))
        sb = ctx.enter_context(tc.tile_pool(name="sb", bufs=2))
        xtp = ctx.enter_context(tc.tile_pool(name="xt", bufs=2))
        spool = ctx.enter_context(tc.tile_pool(name="sp", bufs=2))
        gpool = ctx.enter_context(tc.tile_pool(name="gp", bufs=2))
        opool = ctx.enter_context(tc.tile_pool(name="op", bufs=2))
        statp = ctx.enter_context(tc.tile_pool(name="stat", bufs=1))
        psum_s = ctx.enter_context(tc.tile_pool(name="psum_s", bufs=2, space="PSUM"))
        psum_m = ctx.enter_context(tc.tile_pool(name="psum_m", bufs=2, space="PSUM"))
        dram = ctx.enter_context(tc.tile_pool(name="dram", bufs=2, space="DRAM"))

        ones = consts.tile([128, 512], F32)
        nc.gpsimd.memset(ones[:], 1.0)

        wbotS, wdS, bS = [], [], []
        for li, (d, c) in enumerate(LAYERS):
            wb = consts.tile([d, c], F32, tag=f"wbot{li}")
            nc.sync.dma_start(out=wb[:], in_=wbot[li])
            wbotS.append(wb)
            wdt = consts.tile([d, c], F32, tag=f"wd{li}")
            nc.sync.dma_start(out=wdt[:], in_=wd[li])
            wdS.append(wdt)
            bt = consts.tile([1, c], F32, tag=f"b{li}")
            nc.sync.dma_start(out=bt[:], in_=bias[li])
            bS.append(bt)
        wlS = consts.tile([128, 4, LIN_DIM], F32)
        nc.sync.dma_start(out=wlS[:], in_=wl)

        s1p = statp.tile([128, 32], F32)
        s2p = statp.tile([128, 32], F32)
        mxp = statp.tile([128, 32], F32)

        # k-slot groups for the gather (groups of up to 5 neighbor slots)
        kgroups = []
        k0 = 0
        while k0 < k:
            m = min(5, k - k0)
            kgroups.append((k0, m))
            k0 += m

        for g in range(GPC):
            x0 = sb.tile([3, N], F32, tag="rowbuf")
            nc.sync.dma_start(out=x0[:], in_=posT[g])
            catT = xtp.tile([128, 4, N], F32, tag="catT")
            prev_edge = None

            for li, (d, c) in enumerate(LAYERS):
                ncb = (c + 127) // 128
                if li == 0:
                    xT = x0[:, :]
                else:
                    xT = prev_edge[0:d, 0, :]

                # x2T = 2*x.T ; xsq = x.T^2 ; negsq[j] = -sum_d x[j,d]^2
                x2T = sb.tile([d, N], F32, tag="x2T")
                nc.scalar.mul(out=x2T[:], in_=xT, mul=2.0)
                xsq = sb.tile([d, N], F32, tag="scr")
                nc.scalar.activation(
                    out=xsq[:], in_=xT, func=mybir.ActivationFunctionType.Square
                )
                negsq = sb.tile([1, N], F32, tag="rowbuf")
                for h in range(2):
                    qp = psum_m.tile([128, 512], F32, tag="pm")
                    nc.tensor.matmul(
                        qp[0:1, :],
                        lhsT=ones[0:d, 0:1],
                        rhs=xsq[:, h * 512 : (h + 1) * 512],
                        start=True,
                        stop=True,
                    )
                    nc.scalar.mul(
                        out=negsq[:, h * 512 : (h + 1) * 512], in_=qp[0:1, :], mul=-1.0
                    )

                # BmT = (x @ W_bot).T and AT = (x @ Wd + b).T, feature-major
                bmT = sb.tile([128, 2, N], F32, tag="bmT")
                aT = sb.tile([128, 2, N], F32, tag="aT")
                for cb in range(ncb):
                    cw = min(128, c - cb * 128)
                    cs = slice(cb * 128, cb * 128 + cw)
                    for h in range(2):
                        hs = slice(h * 512, (h + 1) * 512)
                        bp = psum_m.tile([128, 512], F32, tag="pm")
                        nc.tensor.matmul(
                            bp[0:cw, :], lhsT=wbotS[li][:, cs], rhs=xT[:, hs],
                            start=True, stop=True,
                        )
                        nc.scalar.copy(out=bmT[0:cw, cb, hs], in_=bp[0:cw, :])
                        ap_ = psum_m.tile([128, 512], F32, tag="pm")
                        nc.tensor.matmul(
                            ap_[0:cw, :], lhsT=wdS[li][:, cs], rhs=xT[:, hs],
                            start=True, stop=False,
                        )
                        nc.tensor.matmul(
                            ap_[0:cw, :], lhsT=bS[li][:, cs], rhs=ones[0:1, 0:512],
                            start=False, stop=True,
                        )
                        nc.scalar.copy(out=aT[0:cw, cb, hs], in_=ap_[0:cw, :])

                # distance metric + top-k indices
                O = opool.tile([128, kslots, 8], U16, tag="O")
                for q in range(8):
                    xs = xT[:, q * 128 : (q + 1) * 128]
                    sp = psum_s.tile([128, N], F32, tag="s")
                    for h in range(2):
                        sl = slice(h * 512, (h + 1) * 512)
                        nc.tensor.matmul(
                            sp[:, sl], lhsT=xs, rhs=x2T[:, sl], start=True, stop=False
                        )
                        nc.tensor.matmul(
                            sp[:, sl],
                            lhsT=ones[0:1, 0:128],
                            rhs=negsq[:, sl],
                            start=False,
                            stop=True,
                        )
                    S = spool.tile([128, N], F32, tag="S")
                    nc.scalar.copy(out=S[:], in_=sp[:])
                    mx8 = sb.tile([128, 8], F32, tag="mx8")
                    for r in range(rounds):
                        nc.vector.max(out=mx8[:], in_=S[:])
                        nc.vector.max_index(
                            out=O[:, r * 8 : (r + 1) * 8, q], in_max=mx8[:], in_values=S[:]
                        )
                        if r < rounds - 1:
                            nc.vector.match_replace(
                                out=S[:], in_to_replace=mx8[:], in_values=S[:],
                                imm_value=NEG_INF,
                            )

                # index wrap: O[p, kk, q] -> 16-partition-wrapped k-major list.
                # contiguous dump to DRAM, then 8 replication reads whose APs
                # keep 384B-contiguous chunks (2-byte patterns murder the DMA)
                idxD = dram.tile([128, kslots * 8], U16, tag="idxD")
                nc.sync.dma_start(
                    out=idxD[:].rearrange("p (a b) -> p a b", b=8), in_=O[:]
                )
                idx16 = sb.tile([128, kslots * 64], I16, tag="idx16")
                idxDv = (
                    idxD[:].rearrange("(s r) kq -> r kq s", r=16).bitcast(I16)
                )
                for gr in range(8):
                    nc.sync.dma_start(
                        out=idx16[16 * gr : 16 * (gr + 1), :].rearrange(
                            "r (a b) -> r a b", b=8
                        ),
                        in_=idxDv,
                    )

                # gather neighbor BmT columns per k-group, max-reduce on DVE
                edge = gpool.tile([128, 2, N], F32, tag="edge")
                for cb in range(ncb):
                    cw = min(128, c - cb * 128)
                    e = edge[0:cw, cb, :]
                    first = True
                    for (kk0, m) in kgroups:
                        ping = gpool.tile([128, 5 * N], F32, tag="ping")
                        nc.gpsimd.ap_gather(
                            out_ap=ping[0:cw, 0 : m * N],
                            in_ap=bmT[0:cw, cb, :],
                            idxs_ap=idx16[0:cw, kk0 * 64 : (kk0 + m) * 64],
                            channels=cw,
                            num_elems=N,
                            d=1,
                            num_idxs=m * N,
                        )
                        j0 = 0
                        if first:
                            nc.vector.tensor_max(
                                out=e, in0=ping[0:cw, 0:N], in1=ping[0:cw, N : 2 * N]
                            )
                            j0 = 2
                            first = False
                        for j in range(j0, m):
                            nc.vector.tensor_max(
                                out=e, in0=e, in1=ping[0:cw, j * N : (j + 1) * N]
                            )
                    # edge = A.T + max
                    nc.vector.tensor_add(out=e, in0=e, in1=aT[0:cw, cb, :])

                # copy into catT rows (DMA handles partition shifts)
                if li == 0:
                    nc.sync.dma_start(out=catT[0:64, 0, :], in_=edge[0:64, 0, :])
                elif li == 1:
                    nc.sync.dma_start(out=catT[64:128, 0, :], in_=edge[0:64, 0, :])
                elif li == 2:
                    nc.sync.dma_start(out=catT[:, 1, :], in_=edge[:, 0, :])
                else:
                    nc.sync.dma_start(out=catT[:, 2, :], in_=edge[:, 0, :])
                    nc.sync.dma_start(out=catT[:, 3, :], in_=edge[:, 1, :])
                prev_edge = edge

            # linear layer, stats only
            for ct in range(8):
                for nh in range(2):
                    lp = psum_m.tile([128, 512], F32, tag="pm")
                    for kc in range(4):
                        nc.tensor.matmul(
                            lp[:],
                            lhsT=wlS[:, kc, ct * 128 : (ct + 1) * 128],
                            rhs=catT[:, kc, nh * 512 : (nh + 1) * 512],
                            start=(kc == 0),
                            stop=(kc == 3),
                        )
                    col = ct * 4 + nh * 2 + g
                    scr = sb.tile([128, 512], F32, tag="scr")
                    nc.scalar.activation(
                        out=scr[:], in_=lp[:],
                        func=mybir.ActivationFunctionType.Square,
                        accum_out=s2p[:, col : col + 1],
                    )
                    scr2 = sb.tile([128, 512], F32, tag="scr")
                    nc.scalar.activation(
                        out=scr2[:], in_=lp[:],
                        func=mybir.ActivationFunctionType.Copy,
                        accum_out=s1p[:, col : col + 1],
                    )
                    nc.vector.tensor_reduce(
                        out=mxp[:, col : col + 1], in_=lp[:],
                        axis=mybir.AxisListType.X, op=mybir.AluOpType.max,
                    )

        nc.sync.dma_start(out=s1o, in_=s1p[:])
        nc.sync.dma_start(out=s2o, in_=s2p[:])
        nc.sync.dma_start(out=mxo, in_=mxp[:])
    return nc


def _prep_inputs(pos, k, weights):
    """Build the per-core in_maps."""
    pos = np.asarray(pos, dtype=np.float32)
    in_maps = []
    shared = {}
    for li, (d, c) in enumerate(LAYERS):
        W = np.asarray(weights[f"W{li + 1}"], dtype=np.float32)
        b = np.asarray(weights[f"b{li + 1}"], dtype=np.float32)
        wtop, wbot = W[:d], W[d:]
        shared[f"wbot{li}"] = np.ascontiguousarray(wbot)
        shared[f"wd{li}"] = np.ascontiguousarray(wtop - wbot)
        shared[f"b{li}"] = np.ascontiguousarray(b[None, :])
    Wl = np.asarray(weights["Wl"], dtype=np.float32)
    shared["wl"] = np.ascontiguousarray(
        Wl.reshape(4, 128, LIN_DIM).transpose(1, 0, 2)
    )
    for core in range(NCORES):
        m = dict(shared)
        sl = pos[core * GPC : (core + 1) * GPC]  # [GPC, N, 3]
        m["posT"] = np.ascontiguousarray(sl.transpose(0, 2, 1))
        in_maps.append(m)
    return in_maps


def _leaky(x):
    return np.where(x > 0, x, np.float32(NEG) * x).astype(np.float32)


def _host_tail(results, inputs):
    """Combine per-core stats and run the tiny network tail on host."""
    bl = np.asarray(inputs["bl"], np.float32)
    gl = np.asarray(inputs["gl"], np.float32)
    betal = np.asarray(inputs["betal"], np.float32)
    Wm1 = np.asarray(inputs["Wm1"], np.float32)
    bm1 = np.asarray(inputs["bm1"], np.float32)
    gm1 = np.asarray(inputs["gm1"], np.float32)
    betam1 = np.asarray(inputs["betam1"], np.float32)
    Wm2 = np.asarray(inputs["Wm2"], np.float32)
    bm2 = np.asarray(inputs["bm2"], np.float32)

    S1 = np.zeros(LIN_DIM, np.float64)
    S2 = np.zeros(LIN_DIM, np.float64)
    M = np.full((B, LIN_DIM), -np.inf, np.float32)
    for core in range(NCORES):
        r = results[core]
        s1, s2, mx = r["s1"], r["s2"], r["mx"]  # [128, 32]
        for ct in range(8):
            cs = slice(ct * 128, (ct + 1) * 128)
            for nh in range(2):
                for g in range(GPC):
                    col = ct * 4 + nh * 2 + g
                    S1[cs] += s1[:, col].astype(np.float64)
                    S2[cs] += s2[:, col].astype(np.float64)
                    gi = core * GPC + g
                    M[gi, cs] = np.maximum(M[gi, cs], mx[:, col])

    # rows of lin_pre are x + bl; BN over axis 0 is shift invariant in bl only
    # through (x - m); the max-pool then needs the bl-shifted values:
    n_rows = B * N
    mean = (S1 / n_rows).astype(np.float32) + bl
    var = (S2 / n_rows - (S1 / n_rows) ** 2).astype(np.float32)
    Mb = M + bl[None, :]
    rstd = (1.0 / np.sqrt(var + np.float32(EPS))).astype(np.float32)
    pooled = _leaky(gl * (Mb - mean[None, :]) * rstd + betal)

    h = pooled @ Wm1 + bm1
    m1 = h.mean(axis=0, dtype=np.float32)
    v1 = h.var(axis=0, dtype=np.float32)
    h = _leaky(gm1 * (h - m1) * (1.0 / np.sqrt(v1 + np.float32(EPS))) + betam1)
    return (h @ Wm2 + bm2).astype(np.float32)


def run_device(inputs, trace=False):
    k = int(inputs["k"])
    nc = bacc.Bacc(
        "TRN2",
        target_bir_lowering=False,
        debug=False,
        enable_asserts=False,
        num_devices=NCORES,
    )
    _build(nc, k)
    nc.compile()
    in_maps = _prep_inputs(inputs["pos"], k, inputs)
    kw = {}
    if trace:
        _register_ntff_hook()
        kw = dict(trace=True)
    res = run_bass_kernel_spmd(nc, in_maps, list(range(NCORES)), **kw)
    return res


def kernel(**inputs):
    res = run_device(inputs, trace=False)
    return _host_tail(res.results, inputs)


def _register_ntff_hook():
    """Register the axon NTFF profiling hook (antenv.axon_hooks is absent in
    this image; recreate the minimal module so trace=True can capture)."""
    import contextlib
    import ctypes
    import types

    name = "antenv.axon_hooks"
    if name in sys.modules:
        return
    so_path = "/opt/axon/libaxon_pjrt.so"
    mod = types.ModuleType(name)
    _hook_holder = [None]

    try:
        lib = ctypes.CDLL(so_path)
        lib.axon_start_nrt_profile.argtypes = [
            ctypes.POINTER(ctypes.c_int64),
            ctypes.c_size_t,
        ]
        lib.axon_start_nrt_profile.restype = ctypes.c_int64
        lib.axon_stop_nrt_profile.argtypes = [ctypes.c_char_p]
        lib.axon_stop_nrt_profile.restype = ctypes.c_int64

        @contextlib.contextmanager
        def _hook(output_dir, device_ids):
            import jax

            jax.devices()
            if device_ids:
                ids = (ctypes.c_int64 * len(device_ids))(*device_ids)
                rc = lib.axon_start_nrt_profile(ids, len(device_ids))
            else:
                rc = lib.axon_start_nrt_profile(None, 0)
            if rc != 0:
                raise RuntimeError(f"axon_start_nrt_profile rc={rc}")
            try:
                yield
            finally:
                n = lib.axon_stop_nrt_profile(str(output_dir).encode())
                print(f"ntff profile: {n} file(s) -> {output_dir}", file=sys.stderr)

        _hook_holder[0] = _hook
    except Exception as e:  # noqa: BLE001
        print(f"ntff hook unavailable: {e}", file=sys.stderr)

    mod.get_axon_ntff_profile_hook = lambda: _hook_holder[0]
    mod.set_axon_ntff_profile_hook = lambda h: _hook_holder.__setitem__(0, h)
    sys.modules[name] = mod


# revision 13
# speedup vs baseline: 1.2735x; 1.2352x over previous
"""DGCNN forward kernel for Trainium2, data-parallel over 8 NeuronCores.

Strategy (per core, 2 point clouds):
  - EdgeConv layer l: h[n,k,:] = [x_n, x_j - x_n] @ W + b decomposes as
        A[n] = x_n @ (W_top - W_bot) + b          (per-node, PE matmul)
        Bm[j] = x_j @ W_bot                       (per-node, PE matmul)
        out[n] = A[n] + max_{j in knn(n)} Bm[j]
  - kNN ranking metric s[n,j] = 2*x_n.x_j - |x_j|^2 (row-monotone with -dist),
    computed on PE; top-20 per row via DVE max/max_index/match_replace.
  - Neighbor max via 20 chained indirect DMA gathers (DRAM Bm rows) with
    CCE accumulate-max into SBUF.
  - cat -> Wl matmul per 128-channel tile (feature-major); per-channel
    sum / sumsq (ScalarE accum) and per-graph max over nodes (DVE reduce)
    are the only outputs: batch-norm over all B*N rows + leaky relu commute
    with the max-pool (positive scale), so the tiny tail (BN affine on the
    16x1024 pooled matrix + 2-layer classifier) runs on host.
"""

import os
import sys
from contextlib import ExitStack

for _p in ("/opt/trn_rl_repo", "/root/.axon_site/_ro/trn_rl_repo"):
    if os.path.isdir(_p) and _p not in sys.path:
        sys.path.insert(0, _p)

import numpy as np

import concourse.bacc as bacc
import concourse.bass as bass
import concourse.tile as tile
from concourse import mybir
from concourse.bass import IndirectOffsetOnAxis
from concourse.bass_utils import run_bass_kernel_spmd
from concourse.masks import make_identity

F32 = mybir.dt.float32
U32 = mybir.dt.uint32
U16 = mybir.dt.uint16
I16 = mybir.dt.int16

B, N = 16, 1024
NCORES = 8
GPC = B // NCORES  # graphs per core
# (d_in, c_out) for the 4 EdgeConv layers
LAYERS = [(3, 64), (64, 64), (64, 128), (128, 256)]
LIN_DIM, HID = 1024, 512
EPS = 1e-5
NEG = 0.2
NEG_INF = -1.0e30


def _build(nc, k):
    rounds = (k + 7) // 8
    kslots = rounds * 8

    posT = nc.dram_tensor("posT", [GPC, 3, N], F32, kind="ExternalInput").ap()
    wbot, wd, bias = [], [], []
    for li, (d, c) in enumerate(LAYERS):
        wbot.append(nc.dram_tensor(f"wbot{li}", [d, c], F32, kind="ExternalInput").ap())
        wd.append(nc.dram_tensor(f"wd{li}", [d, c], F32, kind="ExternalInput").ap())
        bias.append(nc.dram_tensor(f"b{li}", [1, c], F32, kind="ExternalInput").ap())
    wl = nc.dram_tensor("wl", [128, 4, LIN_DIM], F32, kind="ExternalInput").ap()

    s1o = nc.dram_tensor("s1", [128, 32], F32, kind="ExternalOutput").ap()
    s2o = nc.dram_tensor("s2", [128, 32], F32, kind="ExternalOutput").ap()
    mxo = nc.dram_tensor("mx", [128, 32], F32, kind="ExternalOutput").ap()

    with tile.TileContext(nc) as tc, ExitStack() as ctx:
        consts = ctx.enter_context(tc.tile_pool(name="consts", bufs=1))
        sb = ctx.enter_context(tc.tile_pool(name="sb", bufs=2))
        xtp = ctx.enter_context(tc.tile_pool(name="xt", bufs=2))
        spool = ctx.enter_context(tc.tile_pool(name="sp", bufs=2))
        gpool = ctx.enter_context(tc.tile_pool(name="gp", bufs=2))
        opool = ctx.enter_context(tc.tile_pool(name="op", bufs=2))
        statp = ctx.enter_context(tc.tile_pool(name="stat", bufs=1))
        psum_s = ctx.enter_context(tc.tile_pool(name="psum_s", bufs=2, space="PSUM"))
        psum_m = ctx.enter_context(tc.tile_pool(name="psum_m", bufs=2, space="PSUM"))
        dram = ctx.enter_context(tc.tile_pool(name="dram", bufs=2, space="DRAM"))

        ones = consts.tile([128, 512], F32)
        nc.gpsimd.memset(ones[:], 1.0)

        wbotS, wdS, bS = [], [], []
        for li, (d, c) in enumerate(LAYERS):
            wb = consts.tile([d, c], F32, tag=f"wbot{li}")
            nc.sync.dma_start(out=wb[:], in_=wbot[li])
            wbotS.append(wb)
            wdt = consts.tile([d, c], F32, tag=f"wd{li}")
            nc.sync.dma_start(out=wdt[:], in_=wd[li])
            wdS.append(wdt)
            bt = consts.tile([1, c], F32, tag=f"b{li}")
            nc.sync.dma_start(out=bt[:], in_=bias[li])
            bS.append(bt)
        wlS = consts.tile([128, 4, LIN_DIM], F32)
        nc.sync.dma_start(out=wlS[:], in_=wl)

        s1p = statp.tile([128, 32], F32)
        s2p = statp.tile([128, 32], F32)
        mxp = statp.tile([128, 32], F32)

        for g in range(GPC):
            x0 = sb.tile([3, N], F32, tag="rowbuf")
            nc.sync.dma_start(out=x0[:], in_=posT[g])
            catT = xtp.tile([128, 4, N], F32, tag="catT")
            prev_edge = None

            for li, (d, c) in enumerate(LAYERS):
                ncb = (c + 127) // 128
                if li == 0:
                    xT = x0[:, :]
                else:
                    xT = prev_edge[0:d, 0, :]

                # x2T = 2*x.T ; xsq = x.T^2 ; negsq[j] = -sum_d x[j,d]^2
                x2T = sb.tile([d, N], F32, tag="x2T")
                nc.scalar.mul(out=x2T[:], in_=xT, mul=2.0)
                xsq = sb.tile([d, N], F32, tag="scr")
                nc.scalar.activation(
                    out=xsq[:], in_=xT, func=mybir.ActivationFunctionType.Square
                )
                negsq = sb.tile([1, N], F32, tag="rowbuf")
                for h in range(2):
                    qp = psum_m.tile([128, 512], F32, tag="pm")
                    nc.tensor.matmul(
                        qp[0:1, :],
                        lhsT=ones[0:d, 0:1],
                        rhs=xsq[:, h * 512 : (h + 1) * 512],
                        start=True,
                        stop=True,
                    )
                    nc.scalar.mul(
                        out=negsq[:, h * 512 : (h + 1) * 512], in_=qp[0:1, :], mul=-1.0
                    )

                # BmT = (x @ W_bot).T and AT = (x @ Wd + b).T, feature-major
                bmT = sb.tile([128, 2, N], F32, tag="bmT")
                aT = sb.tile([128, 2, N], F32, tag="aT")
                for cb in range(ncb):
                    cw = min(128, c - cb * 128)
                    cs = slice(cb * 128, cb * 128 + cw)
                    for h in range(2):
                        hs = slice(h * 512, (h + 1) * 512)
                        bp = psum_m.tile([128, 512], F32, tag="pm")
                        nc.tensor.matmul(
                            bp[0:cw, :], lhsT=wbotS[li][:, cs], rhs=xT[:, hs],
                            start=True, stop=True,
                        )
                        nc.scalar.copy(out=bmT[0:cw, cb, hs], in_=bp[0:cw, :])
                        ap_ = psum_m.tile([128, 512], F32, tag="pm")
                        nc.tensor.matmul(
                            ap_[0:cw, :], lhsT=wdS[li][:, cs], rhs=xT[:, hs],
                            start=True, stop=False,
                        )
                        nc.tensor.matmul(
                            ap_[0:cw, :], lhsT=bS[li][:, cs], rhs=ones[0:1, 0:512],
                            start=False, stop=True,
                        )
                        nc.scalar.copy(out=aT[0:cw, cb, hs], in_=ap_[0:cw, :])

                # distance metric + top-k indices
                O = opool.tile([128, kslots, 8], U16, tag="O")
                for q in range(8):
                    xs = xT[:, q * 128 : (q + 1) * 128]
                    sp = psum_s.tile([128, N], F32, tag="s")
                    for h in range(2):
                        sl = slice(h * 512, (h + 1) * 512)
                        nc.tensor.matmul(
                            sp[:, sl], lhsT=xs, rhs=x2T[:, sl], start=True, stop=False
                        )
                        nc.tensor.matmul(
                            sp[:, sl],
                            lhsT=ones[0:1, 0:128],
                            rhs=negsq[:, sl],
                            start=False,
                            stop=True,
                        )
                    S = spool.tile([128, N], F32, tag="S")
                    nc.scalar.copy(out=S[:], in_=sp[:])
                    mx8 = sb.tile([128, 8], F32, tag="mx8")
                    for r in range(rounds):
                        nc.vector.max(out=mx8[:], in_=S[:])
                        nc.vector.max_index(
                            out=O[:, r * 8 : (r + 1) * 8, q], in_max=mx8[:], in_values=S[:]
                        )
                        if r < rounds - 1:
                            nc.vector.match_replace(
                                out=S[:], in_to_replace=mx8[:], in_values=S[:],
                                imm_value=NEG_INF,
                            )

                # index wrap for ap_gather: list position j=16*(s*192+kq)+r
                # holds idx of node n=128q+16s+r, slot kk (kq=kk*8+q). In this
                # word order the 16-partition wrap is a pure partition fold:
                # contiguous 384B chunks on both DMA sides.
                nkq = kslots * 8
                idxD = dram.tile([128, nkq], U16, tag="idxD")
                nc.sync.dma_start(
                    out=idxD[:].rearrange("p (a b) -> p a b", b=8), in_=O[:]
                )
                idx16 = sb.tile([128, kslots * 64], I16, tag="idx16")
                idxDv = idxD[:].rearrange("(s r) kq -> r s kq", r=16).bitcast(I16)
                for gr in range(8):
                    nc.sync.dma_start(
                        out=idx16[16 * gr : 16 * (gr + 1), :].rearrange(
                            "r (s kq) -> r s kq", kq=nkq
                        ),
                        in_=idxDv,
                    )

                # gather neighbor BmT columns per s-group (128 nodes each),
                # then one strided reduce_max over the valid k slots
                edge = gpool.tile([128, 2, N], F32, tag="edge")
                for cb in range(ncb):
                    cw = min(128, c - cb * 128)
                    ev = edge[0:cw, cb, :].rearrange("c (q x) -> c q x", x=128)
                    for s in range(8):
                        ping = gpool.tile([128, kslots * 128], F32, tag="ping")
                        nc.gpsimd.ap_gather(
                            out_ap=ping[0:cw, :],
                            in_ap=bmT[0:cw, cb, :],
                            idxs_ap=idx16[0:cw, s * nkq : (s + 1) * nkq],
                            channels=cw,
                            num_elems=N,
                            d=1,
                            num_idxs=kslots * 128,
                        )
                        pv = ping[0:cw, :].rearrange(
                            "c (kk m) -> c m kk", kk=kslots
                        )[:, :, 0:k]
                        nc.vector.tensor_reduce(
                            out=ev[:, :, 16 * s : 16 * s + 16],
                            in_=pv,
                            axis=mybir.AxisListType.X,
                            op=mybir.AluOpType.max,
                        )
                    # edge = A.T + max
                    nc.vector.tensor_add(
                        out=edge[0:cw, cb, :], in0=edge[0:cw, cb, :],
                        in1=aT[0:cw, cb, :],
                    )

                # copy into catT rows (DMA handles partition shifts)
                if li == 0:
                    nc.sync.dma_start(out=catT[0:64, 0, :], in_=edge[0:64, 0, :])
                elif li == 1:
                    nc.sync.dma_start(out=catT[64:128, 0, :], in_=edge[0:64, 0, :])
                elif li == 2:
                    nc.sync.dma_start(out=catT[:, 1, :], in_=edge[:, 0, :])
                else:
                    nc.sync.dma_start(out=catT[:, 2, :], in_=edge[:, 0, :])
                    nc.sync.dma_start(out=catT[:, 3, :], in_=edge[:, 1, :])
                prev_edge = edge

            # linear layer, stats only
            for ct in range(8):
                for nh in range(2):
                    lp = psum_m.tile([128, 512], F32, tag="pm")
                    for kc in range(4):
                        nc.tensor.matmul(
                            lp[:],
                            lhsT=wlS[:, kc, ct * 128 : (ct + 1) * 128],
                            rhs=catT[:, kc, nh * 512 : (nh + 1) * 512],
                            start=(kc == 0),
                            stop=(kc == 3),
                        )
                    col = ct * 4 + nh * 2 + g
                    scr = sb.tile([128, 512], F32, tag="scr")
                    nc.scalar.activation(
                        out=scr[:], in_=lp[:],
                        func=mybir.ActivationFunctionType.Square,
                        accum_out=s2p[:, col : col + 1],
                    )
                    scr2 = sb.tile([128, 512], F32, tag="scr")
                    nc.scalar.activation(
                        out=scr2[:], in_=lp[:],
                        func=mybir.ActivationFunctionType.Copy,
                        accum_out=s1p[:, col : col + 1],
                    )
                    nc.vector.tensor_reduce(
                        out=mxp[:, col : col + 1], in_=lp[:],
                        axis=mybir.AxisListType.X, op=mybir.AluOpType.max,
                    )

        nc.sync.dma_start(out=s1o, in_=s1p[:])
        nc.sync.dma_start(out=s2o, in_=s2p[:])
        nc.sync.dma_start(out=mxo, in_=mxp[:])
    return nc


def _prep_inputs(pos, k, weights):
    """Build the per-core in_maps."""
    pos = np.asarray(pos, dtype=np.float32)
    in_maps = []
    shared = {}
    for li, (d, c) in enumerate(LAYERS):
        W = np.asarray(weights[f"W{li + 1}"], dtype=np.float32)
        b = np.asarray(weights[f"b{li + 1}"], dtype=np.float32)
        wtop, wbot = W[:d], W[d:]
        shared[f"wbot{li}"] = np.ascontiguousarray(wbot)
        shared[f"wd{li}"] = np.ascontiguousarray(wtop - wbot)
        shared[f"b{li}"] = np.ascontiguousarray(b[None, :])
    Wl = np.asarray(weights["Wl"], dtype=np.float32)
    shared["wl"] = np.ascontiguousarray(
        Wl.reshape(4, 128, LIN_DIM).transpose(1, 0, 2)
    )
    for core in range(NCORES):
        m = dict(shared)
        sl = pos[core * GPC : (core + 1) * GPC]  # [GPC, N, 3]
        m["posT"] = np.ascontiguousarray(sl.transpose(0, 2, 1))
        in_maps.append(m)
    return in_maps


def _leaky(x):
    return np.where(x > 0, x, np.float32(NEG) * x).astype(np.float32)


def _host_tail(results, inputs):
    """Combine per-core stats and run the tiny network tail on host."""
    bl = np.asarray(inputs["bl"], np.float32)
    gl = np.asarray(inputs["gl"], np.float32)
    betal = np.asarray(inputs["betal"], np.float32)
    Wm1 = np.asarray(inputs["Wm1"], np.float32)
    bm1 = np.asarray(inputs["bm1"], np.float32)
    gm1 = np.asarray(inputs["gm1"], np.float32)
    betam1 = np.asarray(inputs["betam1"], np.float32)
    Wm2 = np.asarray(inputs["Wm2"], np.float32)
    bm2 = np.asarray(inputs["bm2"], np.float32)

    S1 = np.zeros(LIN_DIM, np.float64)
    S2 = np.zeros(LIN_DIM, np.float64)
    M = np.full((B, LIN_DIM), -np.inf, np.float32)
    for core in range(NCORES):
        r = results[core]
        s1, s2, mx = r["s1"], r["s2"], r["mx"]  # [128, 32]
        for ct in range(8):
            cs = slice(ct * 128, (ct + 1) * 128)
            for nh in range(2):
                for g in range(GPC):
                    col = ct * 4 + nh * 2 + g
                    S1[cs] += s1[:, col].astype(np.float64)
                    S2[cs] += s2[:, col].astype(np.float64)
                    gi = core * GPC + g
                    M[gi, cs] = np.maximum(M[gi, cs], mx[:, col])

    # rows of lin_pre are x + bl; BN over axis 0 is shift invariant in bl only
    # through (x - m); the max-pool then needs the bl-shifted values:
    n_rows = B * N
    mean = (S1 / n_rows).astype(np.float32) + bl
    var = (S2 / n_rows - (S1 / n_rows) ** 2).astype(np.float32)
    Mb = M + bl[None, :]
    rstd = (1.0 / np.sqrt(var + np.float32(EPS))).astype(np.float32)
    pooled = _leaky(gl * (Mb - mean[None, :]) * rstd + betal)

    h = pooled @ Wm1 + bm1
    m1 = h.mean(axis=0, dtype=np.float32)
    v1 = h.var(axis=0, dtype=np.float32)
    h = _leaky(gm1 * (h - m1) * (1.0 / np.sqrt(v1 + np.float32(EPS))) + betam1)
    return (h @ Wm2 + bm2).astype(np.float32)


def run_device(inputs, trace=False):
    k = int(inputs["k"])
    nc = bacc.Bacc(
        "TRN2",
        target_bir_lowering=False,
        debug=False,
        enable_asserts=False,
        num_devices=NCORES,
    )
    _build(nc, k)
    nc.compile()
    in_maps = _prep_inputs(inputs["pos"], k, inputs)
    kw = {}
    if trace:
        _register_ntff_hook()
        kw = dict(trace=True)
    res = run_bass_kernel_spmd(nc, in_maps, list(range(NCORES)), **kw)
    return res


def kernel(**inputs):
    res = run_device(inputs, trace=False)
    return _host_tail(res.results, inputs)


def _register_ntff_hook():
    """Register the axon NTFF profiling hook (antenv.axon_hooks is absent in
    this image; recreate the minimal module so trace=True can capture)."""
    import contextlib
    import ctypes
    import types

    name = "antenv.axon_hooks"
    if name in sys.modules:
        return
    so_path = "/opt/axon/libaxon_pjrt.so"
    mod = types.ModuleType(name)
    _hook_holder = [None]

    try:
        lib = ctypes.CDLL(so_path)
        lib.axon_start_nrt_profile.argtypes = [
            ctypes.POINTER(ctypes.c_int64),
            ctypes.c_size_t,
        ]
        lib.axon_start_nrt_profile.restype = ctypes.c_int64
        lib.axon_stop_nrt_profile.argtypes = [ctypes.c_char_p]
        lib.axon_stop_nrt_profile.restype = ctypes.c_int64

        @contextlib.contextmanager
        def _hook(output_dir, device_ids):
            import jax

            jax.devices()
            if device_ids:
                ids = (ctypes.c_int64 * len(device_ids))(*device_ids)
                rc = lib.axon_start_nrt_profile(ids, len(device_ids))
            else:
                rc = lib.axon_start_nrt_profile(None, 0)
            if rc != 0:
                raise RuntimeError(f"axon_start_nrt_profile rc={rc}")
            try:
                yield
            finally:
                n = lib.axon_stop_nrt_profile(str(output_dir).encode())
                print(f"ntff profile: {n} file(s) -> {output_dir}", file=sys.stderr)

        _hook_holder[0] = _hook
    except Exception as e:  # noqa: BLE001
        print(f"ntff hook unavailable: {e}", file=sys.stderr)

    mod.get_axon_ntff_profile_hook = lambda: _hook_holder[0]
    mod.set_axon_ntff_profile_hook = lambda h: _hook_holder.__setitem__(0, h)
    sys.modules[name] = mod
